# revision 3
# baseline (speedup 1.0000x reference)
"""GATr→e Trainium2 kernel: 3 GAT blocks over a 100K-node/500K-edge graph.

Strategy: shard NODES across 8 cores. In each GAT block the x_e gather key
equals the segment (scatter) key, so a core that owns a node range and the
edges keyed into it never needs remote data -> zero collectives.

Host prep (per core, per key h/t):
  - nodes relabeled into 128-node tiles (snake-balanced by degree),
  - edges grouped by tile, padded to 128-edge groups (per-tile group counts
    G[j] maxed across cores so one program serves all 8 cores),
  - x_r rows (plus a ones-row for bias) shipped transposed in slot order
    (bf16), one-hot helper ST[n,slot] and local-dst columns shipped too.

Device per tile j (128 nodes, G[j] groups of 128 edges):
  ns = (x_e * a_node).sum(axis=1)                      (DVE)
  per group: e_r^T-chunk matmuls -> PSUM [128e,65]     (PE; col64 = rel_score)
             nsg col = ST_g.T @ ns                     (PE)
  ex = exp(lrelu(nsg + rs))                            (ACT, tile-batched)
  per group: S' = onehot(dst)*ex  (DVE tensor_scalar)
             out_psum += S'.T @ [e_r || ex-col]        (PE, PSUM-accumulated)
  agg = out[:, :64] / (out[:,64]+1e-16); x_e += relu(agg)
Softmax max-subtraction is dropped: logits stay in [-10, 10] here and the
reference's +1e-16 guard is reproduced exactly on the s=0 (degree-0) case.
"""

import math
import numpy as np
import ml_dtypes

BF16 = ml_dtypes.bfloat16

N_NODES = 100000
N_EDGES = 500000
E_HID = 64
IN_DIM = 192
NCORES = 8
NEG_SLOPE = 0.01
P = 128


class Cfg:
    def __init__(self, n_nodes=N_NODES, ncores=NCORES):
        self.n_nodes = n_nodes
        self.ncores = ncores
        self.npc = n_nodes // ncores            # nodes per core
        self.nbins = (self.npc + P - 1) // P    # 128-node tiles per core
        self.block_keys = [0, 1, 0]             # h, t, h


def _snake_bins(deg, nbins):
    """Deal nodes (sorted by degree desc) snake-wise into nbins bins."""
    order = np.argsort(-deg, kind="stable")
    n = len(order)
    rounds = (n + nbins - 1) // nbins
    fwd = np.arange(nbins, dtype=np.int32)
    seq = np.concatenate([fwd if r % 2 == 0 else fwd[::-1] for r in range(rounds)])
    bin_of = np.empty(n, dtype=np.int32)
    bin_of[order] = seq[:n]
    return bin_of


def _host_prep(x_e, x_r, h, t, cfg):
    """Returns (per_core list of dicts of device arrays, G_h, G_t, perm info)."""
    N, NC, NPC, NB = cfg.n_nodes, cfg.ncores, cfg.npc, cfg.nbins
    deg = (np.bincount(h, minlength=N) + np.bincount(t, minlength=N)).astype(np.int64)

    node_new = np.empty(N, dtype=np.int64)  # old -> new local id (within core)
    for c in range(NC):
        lo = c * NPC
        ldeg = deg[lo:lo + NPC]
        bin_of = _snake_bins(ldeg, NB)
        load = np.bincount(bin_of, weights=ldeg.astype(np.float64), minlength=NB)
        border = np.argsort(-load, kind="stable")
        rank = np.empty(NB, dtype=np.int64)
        rank[border] = np.arange(NB)
        nb = rank[bin_of]
        order = np.argsort(nb, kind="stable")
        counts = np.bincount(nb, minlength=NB)
        starts = np.concatenate(([0], np.cumsum(counts)))[:NB]
        newlocal = np.empty(NPC, dtype=np.int64)
        newlocal[order] = np.arange(NPC) - starts[nb[order]] + nb[order] * P
        node_new[lo:lo + NPC] = newlocal

    x_r_b = np.ascontiguousarray(x_r).astype(BF16)

    per_core = [dict() for _ in range(NC)]
    G_prof = {}
    for kname, key in (("h", h), ("t", t)):
        kc = key // NPC
        knew = node_new[key]
        kbin = knew // P
        loads = np.zeros((NC, NB), dtype=np.int64)
        np.add.at(loads, (kc, kbin), 1)
        G = ((loads + P - 1) // P).max(axis=0).astype(np.int64)
        G_prof[kname] = G
        off = P * np.concatenate(([0], np.cumsum(G)))
        S_tot = int(off[-1])
        GT = int(G.sum())
        for c in range(NC):
            ec = np.flatnonzero(kc == c)
            be = kbin[ec]
            dle = (knew[ec] % P).astype(np.int64)
            eo = np.argsort(be, kind="stable")
            be_s, dle_s, eid = be[eo], dle[eo], ec[eo]
            cnt = np.bincount(be_s, minlength=NB)
            bstart = np.concatenate(([0], np.cumsum(cnt)))[:NB]
            slots = off[be_s] + (np.arange(len(eo)) - bstart[be_s])

            rows = np.zeros((S_tot, 193), dtype=BF16)
            rows[slots, :192] = x_r_b[eid]
            rows[slots, 192] = 1
            rT = np.ascontiguousarray(rows.T)
            st = np.zeros((P, S_tot), dtype=BF16)
            st[dle_s, slots] = 1
            dc = np.full(S_tot, -1.0, dtype=np.float32)
            dc[slots] = dle_s
            dc = np.ascontiguousarray(dc.reshape(-1, P).T)
            d = per_core[c]
            xs = np.empty((P, 2 * S_tot), dtype=BF16)
            for j2 in range(NB):
                b0, b1 = int(off[j2]), int(off[j2 + 1])
                xs[:, 2 * b0:2 * b0 + (b1 - b0)] = rT[:128, b0:b1]
                xs[:, 2 * b0 + (b1 - b0):2 * b1] = st[:, b0:b1]
            d["xs_" + kname] = xs
            d["xb_" + kname] = np.ascontiguousarray(rT[128:193])
            d["dc_" + kname] = dc
        assert GT == G.sum()

    for c in range(NC):
        lo = c * NPC
        xe_in = np.zeros((NB * P, E_HID), dtype=np.float32)
        xe_in[node_new[lo:lo + NPC]] = x_e[lo:lo + NPC]
        per_core[c]["xe"] = xe_in
    return per_core, G_prof, node_new


def _weights_arrays(Wr, br, Wr1, br1, Wr2, br2, ah, ah1, at, ar1, ar2, ar3):
    trip = [(Wr, br, ah, ar1), (Wr1, br1, at, ar2), (Wr2, br2, ah1, ar3)]
    wa = np.zeros((3, 128, 66), dtype=np.float32)
    wb = np.zeros((3, 65, 66), dtype=np.float32)
    abc = np.zeros((3, 128, E_HID), dtype=np.float32)
    for b, (W, bias, a_node, a_rel) in enumerate(trip):
        Wp = np.zeros((193, 66), dtype=np.float32)
        Wp[:192, :64] = W.T
        Wp[192, :64] = bias
        Wp[192, 64] = 1.0
        Wp[:192, 65] = W.T @ a_rel
        Wp[192, 65] = float(bias @ a_rel)
        wa[b] = Wp[:128]
        wb[b] = Wp[128:193]
        abc[b] = np.tile(a_node[None, :], (128, 1))
    iota = np.tile(np.arange(128, dtype=np.float32)[None, :], (128, 1))
    # const blobs: bf16 [128, 3*66 + 3*66 + 128], f32 [128, 3*64] (+ dc appended by caller)
    cbf = np.zeros((128, 3 * 66 + 3 * 66 + 128), dtype=BF16)
    for b in range(3):
        cbf[:, b * 66:(b + 1) * 66] = wa[b].astype(BF16)
        cbf[:65, 198 + b * 66:198 + (b + 1) * 66] = wb[b].astype(BF16)
    cbf[:, 396:524] = iota.astype(BF16)
    cf32 = abc.transpose(1, 0, 2).reshape(128, 192)
    return cbf, np.ascontiguousarray(cf32)


def build_program(cfg, G_prof):
    import sys
    if "/opt/trn_rl_repo" not in sys.path:
        sys.path.insert(0, "/opt/trn_rl_repo")
    from concourse import bass, mybir, tile
    from concourse.vector_clock import ScopedClock

    if not getattr(tile.TileContext, "_ant_split_drain", False):
        _orig_dab = tile.TileContext._drain_and_barrier

        def _split_dab(self, tick_clock, wait_clock):
            nc_ = self.nc
            drain_inst = nc_.sync.drain()
            wait_clock.add_sem_waits(
                drain_inst.ins, ScopedClock({None: tick_clock.global_clock})
            )
            si = drain_inst.ins.sync_info
            waits = list(si.on_wait) if si and si.on_wait else []
            if len(waits) > 1:
                upd = list(si.on_update) if si.on_update else []
                drain_inst.ins.sync_info = mybir.SyncInfo(on_wait=waits[:1], on_update=upd)
                for w in waits[1:]:
                    d2 = nc_.sync.drain()
                    d2.ins.sync_info = mybir.SyncInfo(on_wait=[w], on_update=[])
            nc_.all_engine_barrier()
            assert self.sems is not None
            popped = nc_._tile_sem_poison_stack.pop()
            assert popped is self._sem_poison
            nc_.clear_and_free_semaphores(list(self.sems.allocated().values()))
            nc_.all_engine_barrier()

        tile.TileContext._drain_and_barrier = _split_dab
        tile.TileContext._ant_split_drain = True

    NB = cfg.nbins
    nc = bass.Bass()
    f32, bf = mybir.dt.float32, mybir.dt.bfloat16
    A = mybir.AluOpType
    AF = mybir.ActivationFunctionType

    dram = {}
    keys = sorted(set(["h", "t"][kk] for kk in cfg.block_keys))
    off = {}
    GT = {kn: int(G_prof[kn].sum()) for kn in keys}
    for kn in keys:
        G = G_prof[kn]
        S_tot = int(P * G.sum())
        off[kn] = P * np.concatenate(([0], np.cumsum(G)))
        dram["xs_" + kn] = nc.dram_tensor("xs_" + kn, [128, 2 * S_tot], bf, kind="ExternalInput")
        dram["xb_" + kn] = nc.dram_tensor("xb_" + kn, [65, S_tot], bf, kind="ExternalInput")
    CBF_W = 3 * 66 + 3 * 66 + 128
    CF_W = 192 + sum(GT[kn] for kn in keys)
    dram["cbf"] = nc.dram_tensor("cbf", [128, CBF_W], bf, kind="ExternalInput")
    dram["cf32"] = nc.dram_tensor("cf32", [128, CF_W], f32, kind="ExternalInput")
    dram["xe"] = nc.dram_tensor("xe", [NB * P, E_HID], f32, kind="ExternalInput")
    xe_out = nc.dram_tensor("xe_out", [NB * P, E_HID], f32, kind="ExternalOutput")

    dcbase = {}
    acc = 192
    for kn in keys:
        dcbase[kn] = acc
        acc += GT[kn]

    with tile.TileContext(nc) as tc:
        with (
            tc.tile_pool(name="const", bufs=1) as cpool,
            tc.tile_pool(name="ld", bufs=3) as ld,
            tc.tile_pool(name="ersb", bufs=10) as ersb_pool,
            tc.tile_pool(name="work", bufs=4) as work,
            tc.tile_pool(name="spool", bufs=4) as spool,
            tc.tile_pool(name="erps", bufs=3, space="PSUM") as erps_pool,
            tc.tile_pool(name="nsgps", bufs=2, space="PSUM") as nsgps_pool,
            tc.tile_pool(name="outps", bufs=2, space="PSUM") as outps_pool,
        ):
            cbf_sb = cpool.tile([128, CBF_W], bf)
            cf_sb = cpool.tile([128, CF_W], f32)
            xe_sb = cpool.tile([128, NB * E_HID], f32)
            nc.sync.dma_start(out=cbf_sb[:], in_=dram["cbf"][:])
            nc.sync.dma_start(out=cf_sb[:], in_=dram["cf32"][:])
            nc.sync.dma_start(
                out=xe_sb[:].rearrange("p (j d) -> p j d", d=E_HID),
                in_=dram["xe"].rearrange("(j p) d -> p j d", p=P),
            )

            def wa_ap(b):
                return cbf_sb[:, b * 66:(b + 1) * 66]

            def wb_ap(b):
                return cbf_sb[0:65, 198 + b * 66:198 + (b + 1) * 66]

            iota_ap = cbf_sb[:, 396:524]

            # warmup ops observe each const DMA once per engine, so no later
            # compute instruction needs more than one fresh sync wait
            wup = outps_pool.tile([128, 66], f32, tag="outp", name="wup")
            nc.tensor.matmul(wup[0:1, 0:1], iota_ap[:, 0:1], cbf_sb[:, 0:1],
                             start=True, stop=True, skip_group_check=True)
            wupv = work.tile([1, 1], f32, tag="wupv", name="wupv")
            nc.vector.tensor_copy(wupv[:], cf_sb[0:1, 0:1])
            nc.vector.tensor_copy(wupv[:], xe_sb[0:1, 0:1])
            nc.vector.tensor_copy(wupv[:], cbf_sb[0:1, 0:1])
            wupa = work.tile([1, 1], f32, tag="wupa", name="wupa")
            nc.scalar.activation(wupa[:], cbf_sb[0:1, 0:1], AF.Copy)
            nc.scalar.activation(wupa[:], cf_sb[0:1, 0:1], AF.Copy)
            nc.scalar.activation(wupa[:], xe_sb[0:1, 0:1], AF.Copy)

            for b in range(3):
                kn = ["h", "t"][cfg.block_keys[b]]
                G = G_prof[kn]
                for j in range(NB):
                    Gj = int(G[j])
                    if Gj == 0:
                        continue
                    base = int(off[kn][j])
                    gbase = base // P
                    W = Gj * P
                    xs = ld.tile([128, 2 * W], bf, tag="xs", name="xs")
                    xb = ld.tile([65, W], bf, tag="xb", name="xb")
                    nc.sync.dma_start(out=xs[:], in_=dram["xs_" + kn][:, 2 * base:2 * base + 2 * W])
                    nc.sync.dma_start(out=xb[:], in_=dram["xb_" + kn][:, base:base + W])

                    xesl = xe_sb[:, j * E_HID:(j + 1) * E_HID]
                    tmp = work.tile([128, E_HID], f32, tag="tmp", name="tmp")
                    nc.vector.tensor_tensor(tmp[:], xesl, cf_sb[:, b * 64:(b + 1) * 64], op=A.mult)
                    ns_f = work.tile([128, 1], f32, tag="nsf", name="nsf")
                    nc.vector.tensor_reduce(ns_f[:], tmp[:], axis=mybir.AxisListType.X, op=A.add)
                    ns_b = work.tile([128, 1], bf, tag="nsb", name="nsb")
                    nc.vector.tensor_copy(ns_b[:], ns_f[:])

                    nsg = nsgps_pool.tile([128, Gj], f32, tag="nsg", name="nsg")
                    rs = work.tile([128, Gj], f32, tag="rs", name="rs")
                    ers = []
                    for g in range(Gj):
                        sl = slice(g * P, (g + 1) * P)
                        stsl = slice(W + g * P, W + (g + 1) * P)
                        erp = erps_pool.tile([128, 66], f32, tag="erp", name="erp")
                        nc.tensor.matmul(erp[:], xs[:, sl], wa_ap(b), start=True, stop=False,
                                         skip_group_check=True)
                        nc.tensor.matmul(erp[:], xb[:, sl], wb_ap(b), start=False, stop=True,
                                         skip_group_check=True)
                        nc.tensor.matmul(nsg[:, g:g + 1], xs[:, stsl], ns_b[:], start=True,
                                         stop=True, skip_group_check=True)
                        er = ersb_pool.tile([128, 66], bf, tag="er", name="er")
                        nc.scalar.activation(er[:], erp[:], AF.Copy)
                        nc.vector.tensor_copy(rs[:, g:g + 1], erp[:, 65:66])
                        ers.append(er)

                    lg = work.tile([128, Gj], f32, tag="lg", name="lg")
                    nc.vector.tensor_tensor(lg[:], nsg[:], rs[:], op=A.add)
                    lgs = work.tile([128, Gj], f32, tag="lgs", name="lgs")
                    nc.vector.tensor_scalar_mul(lgs[:], lg[:], NEG_SLOPE)
                    lr = work.tile([128, Gj], f32, tag="lr", name="lr")
                    nc.vector.tensor_tensor(lr[:], lg[:], lgs[:], op=A.max)
                    ex = work.tile([128, Gj], f32, tag="ex", name="ex")
                    nc.scalar.activation(ex[:], lr[:], AF.Exp)

                    outp = outps_pool.tile([128, 66], f32, tag="outp", name="outp")
                    for g in range(Gj):
                        dccol = cf_sb[:, dcbase[kn] + gbase + g:dcbase[kn] + gbase + g + 1]
                        sp = spool.tile([128, 128], bf, tag="sp", name="sp")
                        nc.vector.tensor_scalar(sp[:], iota_ap, scalar1=dccol,
                                                scalar2=ex[:, g:g + 1], op0=A.is_equal,
                                                op1=A.mult)
                        nc.tensor.matmul(outp[:, 0:65], sp[:], ers[g][:, 0:65],
                                         start=(g == 0), stop=(g == Gj - 1),
                                         skip_group_check=True)

                    s_eps = work.tile([128, 1], f32, tag="seps", name="seps")
                    nc.vector.tensor_scalar_add(s_eps[:], outp[:, 64:65], 1e-16)
                    rec = work.tile([128, 1], f32, tag="rec", name="rec")
                    nc.vector.reciprocal(rec[:], s_eps[:])
                    rl = work.tile([128, E_HID], f32, tag="rl", name="rl")
                    nc.scalar.activation(rl[:], outp[:, 0:64], AF.Relu, scale=rec[:])
                    nc.vector.tensor_tensor(xesl, xesl, rl[:], op=A.add)

            nc.sync.dma_start(
                out=xe_out.rearrange("(j p) d -> p j d", p=P),
                in_=xe_sb[:].rearrange("p (j d) -> p j d", d=E_HID),
            )
    _fix_sync_waits(nc, mybir)
    return nc, dram




def _fix_sync_waits(nc, mybir):
    """Walrus here allows only ONE sync-wait slot per TPB compute instruction.
    Prune redundant waits via vector-clock transitivity: each instruction's
    observed clock = its engine's running clock + the observed clocks of the
    producers of its waits. A wait already implied by the other kept waits
    (or by the engine clock) is dropped. Own-engine waits fall out for free."""
    import bisect
    sem_hist = {}      # sem -> ([cum values], [inst idx])
    sem_cum = {}
    snap = []          # idx -> observed clock AFTER retire
    eng_obs = {}
    leftover = []

    def merge(dst, src):
        for s, v in src.items():
            if dst.get(s, -1) < v:
                dst[s] = v

    idx = 0
    for bb in nc.m.functions[0].blocks:
        for inst in bb.instructions:
            si = inst.sync_info
            eng = str(inst.engine)
            obs = eng_obs.setdefault(eng, {})
            waits = list(si.on_wait) if si and si.on_wait else []
            covs, prods, simple = [], [], True
            for w in waits:
                if str(w.wait_mode) != "sem-ge-imm" or w.sync_type != "semaphore":
                    simple = False
                    covs.append({}); prods.append(-1)
                    continue
                s, v = str(w.ant_name), w.wait_value
                hist = sem_hist.get(s)
                p = -1
                if hist is not None:
                    q = bisect.bisect_left(hist[0], v)
                    if q < len(hist[0]):
                        p = hist[1][q]
                covs.append(dict(snap[p]) if p >= 0 else {s: v})
                if p >= 0 and covs[-1].get(s, -1) < v:
                    covs[-1][s] = v
                prods.append(p)
            tname = type(inst).__name__
            if simple and len(waits) > 1 and tname != "InstDrain":
                order = sorted(range(len(waits)), key=lambda q2: -prods[q2])
                combined = dict(obs)
                keep = []
                for q2 in order:
                    w = waits[q2]
                    s, v = str(w.ant_name), w.wait_value
                    if combined.get(s, -1) >= v:
                        continue
                    keep.append(w)
                    merge(combined, covs[q2])
                if len(keep) > 1:
                    leftover.append((inst.name, tname, eng,
                                     [str(w)[:70] for w in keep]))
                upd = list(si.on_update) if si.on_update else []
                inst.sync_info = mybir.SyncInfo(on_wait=keep, on_update=upd)
            for c in covs:
                merge(obs, c)
            if si and si.on_update:
                for u in si.on_update:
                    s = str(u.ant_name)
                    if str(u.update_mode) != "sem-inc":
                        sem_hist.pop(s, None)
                        continue
                    cum = sem_cum.get(s, 0) + (u.update_value or 1)
                    sem_cum[s] = cum
                    h2 = sem_hist.setdefault(s, ([], []))
                    h2[0].append(cum)
                    h2[1].append(idx)
                    if obs.get(s, -1) < cum:
                        obs[s] = cum
            snap.append(dict(obs))
            idx += 1
    assert not leftover, f"unpruned multi-wait instrs (n={len(leftover)}): {leftover[:4]}"


def _run(nc, in_maps, ncores, trace=False):
    import sys
    if "/opt/trn_rl_repo" not in sys.path:
        sys.path.insert(0, "/opt/trn_rl_repo")
    from concourse.bass_utils import run_bass_kernel_spmd
    return run_bass_kernel_spmd(nc, in_maps, list(range(ncores)), trace=trace)


def timed_run(nc, in_maps, ncores, iters=6):
    """Time pure device execution: jit without donation, device-resident inputs."""
    import sys, time
    if "/opt/trn_rl_repo" not in sys.path:
        sys.path.insert(0, "/opt/trn_rl_repo")
    import jax
    import numpy as _np
    from concourse import bass2jax, mybir
    from concourse.bass2jax import (_bass_exec_p, install_neuronx_cc_hook,
                                    partition_id_tensor)
    from jax.sharding import Mesh, PartitionSpec, NamedSharding
    from jax.experimental.shard_map import shard_map
    install_neuronx_cc_hook()
    assert nc.dbg_addr is None
    pname = nc.partition_id_tensor.name if nc.partition_id_tensor else None
    in_names, out_names, out_avals, zero_outs = [], [], [], []
    for alloc in nc.m.functions[0].allocations:
        if not isinstance(alloc, mybir.MemoryLocationSet):
            continue
        name = alloc.memorylocations[0].name
        if alloc.kind == "ExternalInput":
            if name != pname:
                in_names.append(name)
        elif alloc.kind == "ExternalOutput":
            shape = tuple(alloc.tensor_shape)
            dtype = mybir.dt.np(alloc.dtype)
            out_names.append(name)
            out_avals.append(jax.core.ShapedArray(shape, dtype))
            zero_outs.append(_np.zeros(shape, dtype))
    n_params = len(in_names)
    all_names = in_names + out_names + ([pname] if pname else [])

    def _body(*args):
        operands = list(args)
        if pname is not None:
            operands.append(partition_id_tensor())
        outs = _bass_exec_p.bind(
            *operands, out_avals=tuple(out_avals), in_names=tuple(all_names),
            out_names=tuple(out_names), lowering_input_output_aliases=(),
            sim_require_finite=True, sim_require_nnan=True, nc=nc)
        return tuple(outs)

    devices = jax.devices()[:ncores]
    mesh = Mesh(_np.asarray(devices), ("core",))
    nsh = NamedSharding(mesh, PartitionSpec("core"))
    in_specs = (PartitionSpec("core"),) * (n_params + len(out_names))
    out_specs = (PartitionSpec("core"),) * len(out_names)
    fn = jax.jit(shard_map(_body, mesh=mesh, in_specs=in_specs,
                           out_specs=out_specs, check_rep=False), keep_unused=True)
    concat = [jax.device_put(_np.concatenate([_np.asarray(in_maps[c][n])
                                              for c in range(ncores)], axis=0), nsh)
              for n in in_names]
    concat += [jax.device_put(_np.concatenate([z] * ncores, axis=0), nsh)
               for z in zero_outs]
    r = fn(*concat)
    jax.block_until_ready(r)
    times = []
    for _ in range(iters):
        t0 = time.perf_counter()
        r = fn(*concat)
        jax.block_until_ready(r)
        times.append(time.perf_counter() - t0)
    return times


def kernel(x_e, x_r, edge_index, rel_size, Wr, br, Wr1, br1, Wr2, br2,
           ah, ah1, at, ar1, ar2, ar3, _trace=False, _cfg=None):
    cfg = _cfg or Cfg()
    x_e = np.asarray(x_e, np.float32)
    x_r = np.asarray(x_r, np.float32)
    ei = np.asarray(edge_index)
    h = ei[0].astype(np.int64)
    t = ei[1].astype(np.int64)
    rs_idx = np.asarray(rel_size).astype(np.int64)
    if not np.array_equal(rs_idx, np.arange(len(rs_idx), dtype=np.int64)):
        x_r = np.ascontiguousarray(np.asarray(x_r)[rs_idx])

    per_core, G_prof, node_new = _host_prep(x_e, x_r, h, t, cfg)
    cbf, cf32 = _weights_arrays(
        np.asarray(Wr, np.float32), np.asarray(br, np.float32),
        np.asarray(Wr1, np.float32), np.asarray(br1, np.float32),
        np.asarray(Wr2, np.float32), np.asarray(br2, np.float32),
        np.asarray(ah, np.float32), np.asarray(ah1, np.float32),
        np.asarray(at, np.float32), np.asarray(ar1, np.float32),
        np.asarray(ar2, np.float32), np.asarray(ar3, np.float32))

    nc, _ = build_program(cfg, G_prof)
    keys = sorted(set(["h", "t"][kk] for kk in cfg.block_keys))
    in_maps = []
    for c in range(cfg.ncores):
        pc = per_core[c]
        m = {"xe": pc["xe"], "cbf": cbf}
        dcs = [cf32] + [pc["dc_" + kn] for kn in keys]
        m["cf32"] = np.ascontiguousarray(np.concatenate(dcs, axis=1))
        for kn in keys:
            m["xs_" + kn] = pc["xs_" + kn]
            m["xb_" + kn] = pc["xb_" + kn]
        in_maps.append(m)
    kernel._last_nc = nc
    kernel._last_in_maps = in_maps
    res = _run(nc, in_maps, cfg.ncores, trace=_trace)

    out = np.empty((cfg.n_nodes, E_HID), dtype=np.float32)
    NPC = cfg.npc
    for c in range(cfg.ncores):
        dev = np.asarray(res.results[c]["xe_out"], np.float32)
        lo = c * NPC
        out[lo:lo + NPC] = dev[node_new[lo:lo + NPC]]
    if _trace:
        kernel._last_result = res
    return out

